# revision 55
# baseline (speedup 1.0000x reference)
"""Trainium2 Bass kernel for LocalSLC GNN message passing.

Computation (per batch b):
    y[b,n,o] = sum_{k,i} bs[n,k] * ws[k,i,o] * x[b, knn_ids[n,k], i]

Shapes: B=16, N=10000, K=16, C_IN=C_OUT=64, fp32.

Strategy (8 NeuronCores; batch packed 8-wide in fp16, nodes split 4-way):
  * Host packs x for batch-group g as xq[n, 512] fp16 =
    [x[8g],...,x[8g+7]] rows (1 KiB).  Core c = 4g+q computes nodes
    [2500q, 2500q+2500) for the 8 batches of group g, gathering from
    the FULL node table.
  * Transpose-mode indirect DMAs (512 indices per call — the hw limit
    for transpose gathers) fetch neighbor rows straight into the
    TRANSPOSED layout zT[128, chunk, 4, 512]: partitions = (2 batches
    x 64 features), free = (chunk, batch-pair c, node).  No PE
    transposes needed.  Four 512-node superblocks use one chunk per k;
    the final 452 nodes are packed flat k-major with no per-k padding
    (14x512 + 1x128 indices instead of 16x512).
  * bs[n,k] scaling: PE rank-1 matmuls (ones[1,128] x bs_row chunks)
    broadcast bs across partitions into PSUM, ACT escapes to fp16 SBUF,
    an in-place DVE multiply per chunk scales zT.
  * Per (c, k): fp16 matmuls with stationary block-diag W2[k] =
    diag(ws[k], ws[k]) accumulate yT[(2 batches x 64 out), nodes] over
    the 16 k's in one PSUM bank; ACT/DVE escape to fp16 and DMA out.
  * Host transposes yT back and casts to fp32.
"""

import numpy as np

import concourse.bass as bass
import concourse.tile as tile
from concourse import bacc, mybir

B, N, K, CI, CO = 16, 10000, 16, 64, 64
NCORES = 8
GROUPS = 2            # batch groups of 8
QUARTER = N // 4      # 2500 nodes per core
SB = 512              # nodes per regular superblock
NSBR = 4              # regular superblocks
REAL = QUARTER - NSBR * SB            # 452 nodes in the final superblock
FLAT = K * REAL                       # 7232 flat columns, k-major
LCH = [512] * (FLAT // 512) + ([FLAT % 512 + 127 & -128] if FLAT % 512 else [])
PFLAT = sum(LCH)                      # 7296 (last chunk padded to 128)
NPAD = 10240          # table rows (N padded)
EW = 8 * CI           # fp16 words per table row (1 KiB)
CC = EW // 128        # 4 batch-pair column groups
# wrapped-idx columns: regular (sb, k) chunks then the flat last-sb chunks
NWCOL = NSBR * K * (SB // 16) + sum(c // 16 for c in LCH)
NBSR = NSBR * K * SB + PFLAT          # bs row length


def _runs(k):
    """Split flat segment [k*REAL, (k+1)*REAL) at 512-column chunk
    boundaries -> list of (chunk, offset-in-chunk, node0, run)."""
    out = []
    pos = k * REAL
    while pos < (k + 1) * REAL:
        j, off = pos // 512, pos % 512
        run = min(512 - off, (k + 1) * REAL - pos)
        out.append((j, off, pos - k * REAL, run))
        pos += run
    return out


def build_program():
    """Build the per-core Bass program (identical on all 8 cores)."""
    nc = bacc.Bacc("TRN2", target_bir_lowering=False, debug=False)
    f16, f32, i16 = mybir.dt.float16, mybir.dt.float32, mybir.dt.int16

    xq = nc.dram_tensor("xq", [NPAD, EW], f16, kind="ExternalInput").ap()
    idsw = nc.dram_tensor("idsw", [128, NWCOL], i16,
                          kind="ExternalInput").ap()
    bsr = nc.dram_tensor("bsr", [1, NBSR], f16, kind="ExternalInput").ap()
    # block-diag weights, i-major so the load uses 4 KiB descriptors
    wts = nc.dram_tensor("w2", [2 * CI, K * 2 * CO], f16,
                         kind="ExternalInput").ap()
    yT = nc.dram_tensor("yT", [CC, 2 * CO, QUARTER], f16,
                        kind="ExternalOutput").ap()

    with tile.TileContext(nc) as tc:
        with (
            tc.tile_pool(name="const", bufs=1) as const_pool,
            tc.tile_pool(name="meta", bufs=4) as meta_pool,
            tc.tile_pool(name="z", bufs=2) as z_pool,
            tc.tile_pool(name="bsb", bufs=2) as bsb_pool,
            tc.tile_pool(name="ysb", bufs=3) as ysb_pool,
            tc.tile_pool(name="bsp", bufs=4, space="PSUM") as bsp_pool,
            tc.tile_pool(name="yp", bufs=1, space="PSUM") as yp_pool,
        ):
            c0 = K * (SB // 16)
            ids_s = const_pool.tile([128, NWCOL], i16)
            nc.sync.dma_start(out=ids_s[:, :c0], in_=idsw[:, :c0])
            ones = const_pool.tile([1, 128], f16)
            nc.vector.memset(ones[:], 1.0)
            w2_s = const_pool.tile([128, K, 2 * CO], f16)
            nc.sync.dma_start(out=w2_s[:].rearrange("i k o -> i (k o)"),
                              in_=wts[:])
            nc.sync.dma_start(out=ids_s[:, c0:], in_=idsw[:, c0:])

            for s in range(NSBR + 1):
                last = s == NSBR
                sbn = REAL if last else SB          # nodes in this sb
                chs = LCH if last else [SB] * K     # gather chunk sizes
                ibase = s * K * (SB // 16)
                bbase = s * K * SB
                zT = z_pool.tile([128, K, CC, SB], f16, tag="z")
                zfl = zT[:].rearrange("p a b c -> p (a b c)")
                bsb = bsb_pool.tile([128, K, SB], f16, tag="bsb")
                bfl = bsb[:].rearrange("p a b -> p (a b)")
                y_ps = [yp_pool.tile([2 * CO, SB], f32, tag=f"y{c}",
                                     name=f"yps{c}")
                        for c in range(CC)]
                bsrows = []
                for half in range(2):
                    hlen = [K // 2 * SB, (PFLAT if last else K * SB)
                            - K // 2 * SB][half]
                    bsrow = meta_pool.tile([1, K // 2 * SB], f16,
                                           tag="bsr", name="bsrow")
                    nc.sync.dma_start(
                        out=bsrow[:1, :hlen],
                        in_=bsr[:1, bbase + half * (K // 2) * SB:
                                bbase + half * (K // 2) * SB + hlen])
                    bsrows.append(bsrow)

                ic = ibase
                fpos = 0
                for j, ch in enumerate(chs):
                    if ch == SB:
                        zout = zT[:, j, :, :]
                    else:
                        zout = zfl[:, j * CC * SB:
                                   j * CC * SB + CC * ch].rearrange(
                            "p (b c) -> p b c", b=CC)
                    nc.gpsimd.dma_gather(
                        out_ap=zout,
                        in_ap=xq[:],
                        idxs_ap=ids_s[:, ic:ic + ch // 16],
                        num_idxs=ch,
                        num_idxs_reg=ch,
                        elem_size=EW,
                        transpose=True,
                    )
                    ic += ch // 16
                    # broadcast bs row across partitions: PE rank-1 matmul
                    half = fpos // (K // 2 * SB)
                    hoff = fpos % (K // 2 * SB)
                    bs_ps = bsp_pool.tile([128, SB], f32, tag="bsp")
                    nc.tensor.matmul(
                        bs_ps[:, :ch],
                        lhsT=ones[:],
                        rhs=bsrows[half][:1, hoff:hoff + ch],
                        start=True,
                        stop=True,
                    )
                    bslice = bfl[:, j * SB:j * SB + ch]
                    nc.scalar.copy(out=bslice, in_=bs_ps[:, :ch])
                    if j >= len(chs) - 2:
                        for c in range(CC):
                            nc.vector.tensor_mul(
                                zout[:, c, :], zout[:, c, :], bslice)
                    else:
                        nc.vector.tensor_mul(
                            zout, zout,
                            bslice.unsqueeze(1).broadcast_to([128, CC, ch]))
                    fpos += ch
                    if not last:
                        k = j
                        for c in range(CC):
                            nc.tensor.matmul(
                                y_ps[c][:],
                                lhsT=w2_s[:, k, :],
                                rhs=zT[:, k, c, :],
                                start=(k == 0),
                                stop=(k == K - 1),
                            )
                if last:
                    # flat layout: k segments cross chunk boundaries
                    for k in range(K):
                        for (j, off, n0, run) in _runs(k):
                            if chs[j] == SB:
                                rhss = [zT[:, j, c, off:off + run]
                                        for c in range(CC)]
                            else:
                                base = j * CC * SB
                                rhss = [zfl[:, base + c * chs[j] + off:
                                            base + c * chs[j] + off + run]
                                        for c in range(CC)]
                            for c in range(CC):
                                nc.tensor.matmul(
                                    y_ps[c][:, n0:n0 + run],
                                    lhsT=w2_s[:, k, :],
                                    rhs=rhss[c],
                                    start=(k == 0),
                                    stop=(k == K - 1),
                                )
                for cp in range(CC // 2):
                    y_sb = ysb_pool.tile([2 * CO, 2, SB], f16, tag="ysb",
                                         name="ysb")
                    nc.scalar.copy(out=y_sb[:, 0, :sbn],
                                   in_=y_ps[2 * cp][:, :sbn])
                    nc.vector.tensor_copy(out=y_sb[:, 1, :sbn],
                                          in_=y_ps[2 * cp + 1][:, :sbn])
                    if last:
                        nc.sync.dma_start(
                            out=yT[2 * cp:2 * cp + 2, :,
                                   s * SB:s * SB + sbn].rearrange(
                                "c p n -> p c n"),
                            in_=y_sb[:, :, :sbn])
                    else:
                        for h in range(2):
                            nc.sync.dma_start(
                                out=yT[2 * cp + h, :, s * SB:s * SB + sbn],
                                in_=y_sb[:, h, :sbn])

    nc.compile()
    return nc


_CACHE = {}


def _get_program():
    if "nc" not in _CACHE:
        _CACHE["nc"] = build_program()
    return _CACHE["nc"]


def _wrap(chunk):
    """Wrap a 1-D idx chunk (len % 16 == 0): w[p, t] = chunk[t*16 + p%16],
    replicated across the 8 q7 cores -> [128, len//16] int16."""
    w = chunk.reshape(-1, 16).T
    return np.tile(w, (8, 1)).astype(np.int16)


def _pack_inputs(x, knn_ids, bs, ws):
    """Host-side packing into per-core input maps."""
    # fp16 table per batch group: [NPAD, 512] rows of 8 batches x 64 feats
    xqs = []
    for g in range(GROUPS):
        xq = np.zeros((NPAD, EW), np.float16)
        for b in range(8):
            xq[:N, b * CI:(b + 1) * CI] = x[8 * g + b]
        xqs.append(xq)

    # per-quarter wrapped indices and k-major bs rows
    idsw_q, bsr_q = [], []
    for q in range(4):
        n0 = q * QUARTER
        wcols, bsflat = [], []
        for s in range(NSBR):
            lo = n0 + s * SB
            ids_sb = knn_ids[lo:lo + SB].T.astype(np.int32)  # [K, SB]
            for k in range(K):
                wcols.append(_wrap(ids_sb[k]))
            bsflat.append(bs[lo:lo + SB].T.reshape(-1))
        # final superblock: flat k-major, padded to PFLAT
        lo = n0 + NSBR * SB
        idsl = np.zeros(PFLAT, np.int32)
        idsl[:FLAT] = knn_ids[lo:lo + REAL].T.reshape(-1)
        bsl = np.zeros(PFLAT, np.float32)
        bsl[:FLAT] = bs[lo:lo + REAL].T.reshape(-1)
        p = 0
        for ch in LCH:
            wcols.append(_wrap(idsl[p:p + ch]))
            p += ch
        bsflat.append(bsl)
        idsw_q.append(np.ascontiguousarray(
            np.concatenate(wcols, axis=1).astype(np.int16)))
        bsr_q.append(np.concatenate(bsflat).astype(
            np.float16).reshape(1, NBSR))

    # block-diag W2[k] = diag(ws[k], ws[k]), stored i-major: [i, (k, o)]
    w2k = np.zeros((K, 2 * CI, 2 * CO), np.float32)
    w2k[:, :CI, :CO] = ws
    w2k[:, CI:, CO:] = ws
    w2 = np.ascontiguousarray(
        w2k.transpose(1, 0, 2).reshape(2 * CI, K * 2 * CO).astype(np.float16))

    in_maps = []
    for c in range(NCORES):
        g, q = c // 4, c % 4
        in_maps.append({"xq": xqs[g], "idsw": idsw_q[q], "bsr": bsr_q[q],
                        "w2": w2})
    return in_maps


def kernel(x, knn_ids, bs, ws):
    from concourse import bass_utils

    x = np.asarray(x, np.float32)
    knn_ids = np.asarray(knn_ids, np.int32)
    bs = np.asarray(bs, np.float32)
    ws = np.asarray(ws, np.float32)

    nc = _get_program()
    in_maps = _pack_inputs(x, knn_ids, bs, ws)
    try:
        res = bass_utils.run_bass_kernel_spmd(
            nc, in_maps, core_ids=list(range(NCORES))
        )
    except Exception:
        # one retry: a crashed previous tenant can leave a core in
        # NRT_EXEC_UNIT_UNRECOVERABLE until the next nrt_init resets it
        res = bass_utils.run_bass_kernel_spmd(
            nc, in_maps, core_ids=list(range(NCORES))
        )

    y = np.empty((B, N, CO), np.float32)
    for c in range(NCORES):
        g, q = c // 4, c % 4
        n0 = q * QUARTER
        yt = res.results[c]["yT"]  # [CC, 128, QUARTER] f16
        for cc in range(CC):
            for p in range(2):
                b = 8 * g + 2 * cc + p
                y[b, n0:n0 + QUARTER] = (
                    yt[cc, p * CO:(p + 1) * CO, :].T.astype(np.float32))
    return y
